# revision 1
# baseline (speedup 1.0000x reference)
"""Multi-head attention (B=4, G=2048, C=1024, H=16) on 8 TRN2 NeuronCores.

Sharding: (batch x head-half). Core c handles batch c//2 and an 8-head
slice (c%2). Each core computes its heads' q/k/v projections, full
softmax attention, and a partial output projection over its 512
channels; the host sums core pairs and adds the bias.

Device kernel (Bass/Tile, all matmuls as float32r):
  - qT/kT in [o, g] layout straight from the projection matmuls (x is
    pre-transposed on the host, so no on-device transposes anywhere).
  - scores are computed transposed ([k, q]); softmax needs no max
    subtraction (scores are small by construction) and the denominator
    comes for free from a ones-column appended to v.
  - exp on ScalarE fused with the 1/sqrt(d) scale.
"""

from contextlib import ExitStack

import numpy as np

import concourse.bass as bass
import concourse.tile as tile
from concourse import mybir
from concourse.bass_utils import run_bass_kernel_spmd
from concourse.vector_clock import ScopedClock, VectorClock
from concourse.tile_sem_assignment import N_PROCS

F32 = mybir.dt.float32
F32R = mybir.dt.float32r

B, G, C, H = 4, 2048, 1024, 16
N_CORES = 8
H_LOC = H // 2
O_LOC = H_LOC * 64


class SplitDrainTileContext(tile.TileContext):
    """Tail drain limited to one sync wait per instruction.

    This environment's walrus rejects >1 sync wait per instruction, so
    wait on each outstanding proc tick with its own NOP first and emit
    the drain bare.
    """

    def _drain_and_barrier(self, tick_clock, wait_clock):
        g = tick_clock.global_clock
        for p in range(N_PROCS):
            if g[p] > 0:
                nop = self.nc.sync.nop(nofuse=True)
                partial = VectorClock([g[q] if q == p else 0 for q in range(N_PROCS)])
                wait_clock.add_sem_waits(nop.ins, ScopedClock({None: partial}))
        self.nc.sync.drain()
        self.nc.all_engine_barrier()
        assert self.sems is not None
        popped = self.nc._tile_sem_poison_stack.pop()
        assert popped is self._sem_poison
        self.nc.clear_and_free_semaphores(list(self.sems.allocated().values()))
        self.nc.all_engine_barrier()


def split_multi_waits(nc):
    """Hoist extra sync waits onto NOPs before each offending instruction
    (this walrus accepts at most one sync wait per instruction)."""
    n_split = 0
    for f in nc.m.functions:
        for bb in f.blocks:
            insts = bb.instructions
            out = []
            for inst in insts:
                si = inst.sync_info
                waits = list(si.on_wait) if si and si.on_wait else []
                if len(waits) > 1:
                    for w in waits[:-1]:
                        nop = mybir.InstNoOp(
                            name=f"{inst.name}_w{n_split}",
                            engine=inst.engine,
                            ins=[],
                            outs=[],
                            sync_info=mybir.SyncInfo(on_wait=[w], on_update=[]),
                        )
                        out.append(nop)
                        n_split += 1
                    inst.sync_info = mybir.SyncInfo(
                        on_wait=[waits[-1]],
                        on_update=list(si.on_update) if si.on_update else [],
                    )
                out.append(inst)
            if len(out) != len(insts):
                bb.instructions[:] = out
    return n_split


def build_program():
    D = 64
    scale = D ** -0.5
    CC = C // 128
    OC = O_LOC // 128
    GC = G // 128
    KC = G // 128

    nc = bass.Bass()
    xT = nc.declare_dram_parameter("xT", [C, G], F32, isOutput=False)
    wqT = nc.declare_dram_parameter("wqT", [C, O_LOC], F32, isOutput=False)
    wkT = nc.declare_dram_parameter("wkT", [C, O_LOC], F32, isOutput=False)
    wvT = nc.declare_dram_parameter("wvT", [C, O_LOC], F32, isOutput=False)
    wpT = nc.declare_dram_parameter("wpT", [O_LOC, C], F32, isOutput=False)
    out_p = nc.declare_dram_parameter("out_p", [G, C], F32, isOutput=True)

    rcp_dram = nc.dram_tensor("rcp_scratch", [H_LOC, G], F32)

    with SplitDrainTileContext(nc) as tc, ExitStack() as ctx:
        persist = ctx.enter_context(tc.tile_pool(name="persist", bufs=1))
        qT_t = [persist.tile([128, G], F32R, name=f"qT{t}", tag=f"qT{t}") for t in range(OC)]
        kT_t = [persist.tile([128, G], F32R, name=f"kT{t}", tag=f"kT{t}") for t in range(OC)]
        v_sb = persist.tile([128, GC, H_LOC, 65], F32R, name="v_sb", tag="v_sb")

        # ---------------- phase 1: QKV projections ----------------
        with tc.tile_pool(name="ph1_w", bufs=1) as wpool, \
             tc.tile_pool(name="ph1_x", bufs=1) as xpool, \
             tc.tile_pool(name="ph1_ps", bufs=4, space="PSUM") as ps1:
            wq_sb = wpool.tile([128, CC, O_LOC], F32R, name="wq_sb", tag="wq")
            wk_sb = wpool.tile([128, CC, O_LOC], F32R, name="wk_sb", tag="wk")
            wv_sb = wpool.tile([128, CC, O_LOC], F32R, name="wv_sb", tag="wv")
            nc.sync.dma_start(out=wq_sb[:], in_=wqT.rearrange("(cc p) o -> p cc o", p=128).bitcast(F32R))
            nc.sync.dma_start(out=wk_sb[:], in_=wkT.rearrange("(cc p) o -> p cc o", p=128).bitcast(F32R))
            nc.sync.dma_start(out=wv_sb[:], in_=wvT.rearrange("(cc p) o -> p cc o", p=128).bitcast(F32R))
            ones_t = wpool.tile([128, GC, H_LOC, 1], F32, name="ones_t", tag="ones")
            nc.vector.memset(ones_t[:], 1.0)
            nc.vector.tensor_copy(out=v_sb[:, :, :, 64:65], in_=ones_t[:])

            GH = G // 2
            for gh in range(2):
                xh = xpool.tile([128, CC, GH], F32R, name="xh", tag="xh")
                nc.sync.dma_start(
                    out=xh[:],
                    in_=xT[:, gh * GH:(gh + 1) * GH].rearrange("(cc p) g -> p cc g", p=128).bitcast(F32R),
                )
                for w_sb, dst in ((wq_sb, qT_t), (wk_sb, kT_t)):
                    for oc in range(OC):
                        for z in range(GH // 512):
                            ps = ps1.tile([128, 512], F32, name="ps_qk", tag="ps_qk")
                            for cc in range(CC):
                                nc.tensor.matmul(
                                    ps[:],
                                    w_sb[:, cc, oc * 128:(oc + 1) * 128],
                                    xh[:, cc, z * 512:(z + 1) * 512],
                                    start=(cc == 0), stop=(cc == CC - 1),
                                )
                            nc.scalar.copy(
                                out=dst[oc][:, gh * GH + z * 512: gh * GH + (z + 1) * 512],
                                in_=ps[:],
                            )
                for gc8 in range(GH // 128):
                    gc = gh * (GH // 128) + gc8
                    ps = ps1.tile([128, O_LOC], F32, name="ps_v", tag="ps_v")
                    for cc in range(CC):
                        nc.tensor.matmul(
                            ps[:],
                            xh[:, cc, gc8 * 128:(gc8 + 1) * 128],
                            wv_sb[:, cc, :],
                            start=(cc == 0), stop=(cc == CC - 1),
                        )
                    nc.vector.tensor_copy(out=v_sb[:, gc, :, 0:64], in_=ps[:])

        # ---------------- phase 2+3 persistent SBUF ----------------
        p23 = ctx.enter_context(tc.tile_pool(name="p23", bufs=1))
        oT_t = [p23.tile([128, G], F32R, name=f"oT{t}", tag=f"oT{t}") for t in range(OC)]
        wp_sb = p23.tile([128, O_LOC // 128, C], F32R, name="wp_sb", tag="wp")
        nc.sync.dma_start(out=wp_sb[:], in_=wpT.rearrange("(ct p) o -> p ct o", p=128).bitcast(F32R))

        # ---------------- phase 2: attention ----------------
        with tc.tile_pool(name="ph2_exp", bufs=3) as epool, \
             tc.tile_pool(name="ph2_den", bufs=2) as dpool, \
             tc.tile_pool(name="ph2_bc", bufs=2) as bcpool, \
             tc.tile_pool(name="ph2_sc", bufs=2, space="PSUM") as scps, \
             tc.tile_pool(name="ph2_av", bufs=1, space="PSUM") as avps:
            for h in range(H_LOC):
                t, base = h // 2, (h % 2) * 64
                av = avps.tile([65, G], F32, name="av", tag="av")
                for kc in range(KC):
                    for qh in range(G // 1024):
                        sc = scps.tile([128, 1024], F32, name="sc", tag="sc")
                        for z in range(2):
                            nc.tensor.matmul(
                                sc[:, z * 512:(z + 1) * 512],
                                kT_t[t][base:base + D, kc * 128:(kc + 1) * 128],
                                qT_t[t][base:base + D,
                                        qh * 1024 + z * 512: qh * 1024 + (z + 1) * 512],
                                start=True, stop=True,
                            )
                        ex = epool.tile([128, 1024], F32R, name="ex", tag="ex")
                        nc.scalar.activation(
                            out=ex[:], in_=sc[:],
                            func=mybir.ActivationFunctionType.Exp, scale=scale,
                        )
                        for z in range(2):
                            nc.tensor.matmul(
                                av[:, qh * 1024 + z * 512: qh * 1024 + (z + 1) * 512],
                                v_sb[:, kc, h, :],
                                ex[:, z * 512:(z + 1) * 512],
                                start=(kc == 0), stop=(kc == KC - 1),
                            )
                den_row = dpool.tile([1, G], F32, name="den_row", tag="den_row")
                nc.vector.tensor_copy(out=den_row[:], in_=av[64:65, :])
                den_h = dpool.tile([128, G // 128], F32, name="den_h", tag="den_h")
                nc.sync.dma_start(
                    out=den_h[:],
                    in_=bass.AP(tensor=den_row.tensor, offset=den_row.offset,
                                ap=[[1, 1], [G // 128, 128], [1, G // 128]]),
                )
                nc.vector.reciprocal(out=den_h[:], in_=den_h[:])
                nc.sync.dma_start(out=rcp_dram[h, :], in_=den_h[:])
                bc = bcpool.tile([64, G], F32, name="bc", tag="bc")
                row = rcp_dram[h, :]
                nc.sync.dma_start(
                    out=bc[:],
                    in_=bass.AP(tensor=row.tensor, offset=row.offset,
                                ap=[[0, 64], [1, G]]),
                )
                nc.vector.tensor_mul(
                    out=oT_t[t][base:base + D, :], in0=av[0:64, :], in1=bc[:],
                )

        # ---------------- phase 3: output projection ----------------
        with tc.tile_pool(name="ph3_st", bufs=3) as stpool, \
             tc.tile_pool(name="ph3_ps", bufs=2, space="PSUM") as ps3:
            CT = O_LOC // 128
            for gc in range(GC):
                po = ps3.tile([128, C], F32, name="po", tag="po")
                for z in range(C // 512):
                    for ct in range(CT):
                        nc.tensor.matmul(
                            po[:, z * 512:(z + 1) * 512],
                            oT_t[ct][:, gc * 128:(gc + 1) * 128],
                            wp_sb[:, ct, z * 512:(z + 1) * 512],
                            start=(ct == 0), stop=(ct == CT - 1),
                        )
                st = stpool.tile([128, C], F32, name="st", tag="st")
                nc.scalar.copy(out=st[:], in_=po[:])
                nc.sync.dma_start(out=out_p[gc * 128:(gc + 1) * 128, :], in_=st[:])

    split_multi_waits(nc)
    return nc


_CACHE = {}


def make_in_maps(x, Wq, Wk, Wv, Wp):
    WqT, WkT, WvT, WpT = Wq.T, Wk.T, Wv.T, Wp.T
    in_maps = []
    for core in range(N_CORES):
        b, s = core // 2, core % 2
        osl = slice(s * O_LOC, (s + 1) * O_LOC)
        in_maps.append({
            "xT": np.ascontiguousarray(x[b].T),
            "wqT": np.ascontiguousarray(WqT[:, osl]),
            "wkT": np.ascontiguousarray(WkT[:, osl]),
            "wvT": np.ascontiguousarray(WvT[:, osl]),
            "wpT": np.ascontiguousarray(WpT[osl, :]),
        })
    return in_maps


def kernel(x, Wq, Wk, Wv, Wp, bp):
    x = np.ascontiguousarray(np.asarray(x, dtype=np.float32))
    in_maps = make_in_maps(x, np.asarray(Wq), np.asarray(Wk), np.asarray(Wv),
                           np.asarray(Wp))
    if "nc" not in _CACHE:
        _CACHE["nc"] = build_program()
    res = run_bass_kernel_spmd(_CACHE["nc"], in_maps, list(range(N_CORES)))
    out = np.zeros((B, G, C), np.float32)
    bp = np.asarray(bp, dtype=np.float32)
    for b in range(B):
        out[b] = res.results[2 * b]["out_p"] + res.results[2 * b + 1]["out_p"] + bp
    return out

